# revision 86
# baseline (speedup 1.0000x reference)
"""Trainium2 Bass kernel for GQA attention (RoPE + causal) + output projection.

Sharding: (batch, head-half) across 8 cores. Core c handles batch c//2 and
q-heads [8*(c%2), 8*(c%2)+8) with kv-heads {2*(c%2), 2*(c%2)+1}. Each core
writes ONE transposed partial output [D, S] in bf16 (all 8 of its heads
accumulated on-chip); the host sums the two partials per batch and
transposes back.

Engine plan (per core):
- PE: projections (bf16), rope pair-swap (bf16), scores/AV (bf16), softmax
  denominator via all-ones [128,128] stationary (produces the denominator
  broadcast across partitions in one accumulation chain; accumulated
  directly from the prob tiles in the PE-idle first attention set), output
  projection. Attention runs qc-outer so the output projection accumulates
  all 8 heads in a single PSUM chain (one output write, half the DRAM).
- Scalar (Act): rope PSUM->SBUF pair copies, exp on score PAIRS ([128,1024]
  across two PSUM banks; the diagonal-most pair is split into two narrow
  activations to skip sub-band columns), quad vn copies, half the P3
  copies. Act/DVE instructions carry ~350 cycles fixed cost, so copies are
  batched as wide as PSUM allows.
- DVE: rope swap-mult + final add (bf16 2x), causal mask as ONE full-width
  multiply per band pair, softmax partial-sum adds, reciprocal_approx_fast
  on the broadcast denominator, final normalize, half the P3 copies.
- Pool (gpsimd): rope mult (SBUF-only; Pool cannot touch PSUM), const/wk/wv
  /wo-fetch DMAs.
- DMA: all DRAM layouts are partition-major so every transfer moves
  contiguous multi-KB per-partition lines; startup loads are spread across
  the Sync/Act/Pool rings with fine granularity ordered by first use.
"""

import math
from contextlib import ExitStack
from dataclasses import dataclass

import numpy as np

import concourse.bass as bass
import concourse.tile as tile
from concourse import bacc, mybir
from concourse.bass_utils import run_bass_kernel_spmd

F32 = mybir.dt.float32
F32R = mybir.dt.float32r
BF16 = mybir.dt.bfloat16
AF = mybir.ActivationFunctionType
MUL = mybir.AluOpType.mult
ADD = mybir.AluOpType.add


@dataclass(frozen=True)
class Cfg:
    B: int = 4          # batch
    S: int = 2048       # sequence length
    D: int = 2048       # model dim
    HQC: int = 8        # q-heads per core
    HD: int = 128       # head dim
    QCH: int = 512      # chunk (matmul moving free dim)

    @property
    def DT(self):
        return self.D // 128   # d-tiles

    @property
    def KT(self):
        return self.S // 128   # 128-row tiles along S

    @property
    def NQC(self):
        return self.S // self.QCH  # q-chunks

    @property
    def RB(self):
        return self.QCH // 128     # band tiles per q-chunk


def r(ap):
    """View an fp32 AP as float32r for full-rate PE matmuls."""
    return ap.bitcast(F32R)


def build_program(cfg: Cfg):
    c = cfg
    assert c.HD == 128 and c.HQC == 8 and c.RB == 4
    nc = bacc.Bacc("TRN2", target_bir_lowering=False, debug=False)

    # Partition-major DRAM layouts: every DMA below moves contiguous
    # multi-KB per-partition lines (full HBM bandwidth, single descriptor).
    xt_d = nc.dram_tensor("xt", [c.NQC, c.DT // 4, 128, 4, c.QCH], BF16,
                          kind="ExternalInput")
    wq_d = nc.dram_tensor("wq", [2, 128, c.DT, 4 * c.HD], BF16, kind="ExternalInput")
    wk_d = nc.dram_tensor("wk", [128, c.DT, 2 * c.HD], BF16, kind="ExternalInput")
    wv_d = nc.dram_tensor("wv", [128, c.DT, 2 * c.HD], BF16, kind="ExternalInput")
    wo_d = nc.dram_tensor("wo", [c.DT, 128, 8 * c.HD], BF16, kind="ExternalInput")
    ra_d = nc.dram_tensor("ra", [c.HD, c.S], BF16, kind="ExternalInput")
    rb_d = nc.dram_tensor("rb", [c.HD, c.S], BF16, kind="ExternalInput")
    cm_d = nc.dram_tensor("cm", [128, c.RB * c.QCH], BF16, kind="ExternalInput")
    pm_d = nc.dram_tensor("pm", [128, 128], BF16, kind="ExternalInput")
    idn_d = nc.dram_tensor("idn", [128, 128], BF16, kind="ExternalInput")
    ones_d = nc.dram_tensor("ones", [128, 128], BF16, kind="ExternalInput")
    out_d = nc.dram_tensor("partialT", [c.D, c.S], BF16, kind="ExternalOutput")

    scale = 1.0 / math.sqrt(c.HD)

    with tile.TileContext(nc) as tc, ExitStack() as ctx:
        ctx.enter_context(nc.allow_low_precision("bf16 internals; tol 2e-2"))
        const = ctx.enter_context(tc.tile_pool(name="const", bufs=1))
        wp = ctx.enter_context(tc.tile_pool(name="wp", bufs=1))
        wop = ctx.enter_context(tc.tile_pool(name="wop", bufs=4))
        xp = ctx.enter_context(tc.tile_pool(name="xp", bufs=3))
        qkp = ctx.enter_context(tc.tile_pool(name="qkp", bufs=1))
        rtp = ctx.enter_context(tc.tile_pool(name="rtp", bufs=1))
        ptp = ctx.enter_context(tc.tile_pool(name="ptp", bufs=4))
        rp = ctx.enter_context(tc.tile_pool(name="rp", bufs=2))
        ocp = ctx.enter_context(tc.tile_pool(name="ocp", bufs=4))
        ps = ctx.enter_context(
            tc.tile_pool(name="ps", bufs=1, space=bass.MemorySpace.PSUM)
        )

        # PSUM tags (8 banks): stp2 [128,1024] x2 = 4 banks (score pairs, P1 Q
        # accumulators), ot [128,512] x2 = 2 banks (AV accumulators, P1 K/V
        # accumulators, half the P3 endgame), zz [128,512] x2 = 2 banks (rope
        # pair-swap, V transposes, denominator broadcast, P3 accumulators).
        def p_stp2(name):
            return ps.tile([128, 2 * c.QCH], F32, name=name, tag="stp2", bufs=2)

        def p_ot(name):
            return ps.tile([128, c.QCH], F32, name=name, tag="ot", bufs=2)

        def p_zz(name, shape=None, dtype=F32):
            return ps.tile(shape or [128, c.QCH], dtype, name=name, tag="zz",
                           bufs=2)

        # ---- constants: small ones ride the Pool DMA queue so the Sync
        # queue's startup slots go to the weights/x the first matmuls need.
        # The big rope tables are deferred a few d-tiles so their transfers
        # don't compete with the startup-critical weight/x loads.
        # PE warm-up: the clock ramps from idle over the first microseconds;
        # burn discarded matmuls on scratch SBUF during the startup barrier /
        # first-DMA window so real work starts at full clock. The scratch
        # tile is never written; the PSUM result is never read.
        warm_sb = const.tile([128, c.QCH], BF16, name="warm_sb")
        nc.gpsimd.memset(warm_sb[:], 1.0)
        warm_ps = ps.tile([128, c.QCH], F32, name="warm_ps", tag="zz", bufs=2)
        for wi in range(12):
            nc.tensor.matmul(warm_ps[:], warm_sb[:, :128], warm_sb[:],
                             start=(wi == 0), stop=(wi == 11))

        ra_sb = const.tile([128, c.S], BF16, name="ra_sb")
        rb_sb = const.tile([128, c.S], BF16, name="rb_sb")
        cm_sb = const.tile([128, c.RB * c.QCH], BF16, name="cm_sb")
        pm_sb = const.tile([128, 128], BF16, name="pm_sb")
        idn_sb = const.tile([128, 128], BF16, name="idn_sb")
        ones_sb = const.tile([128, 128], BF16, name="ones_sb")
        # (pm/idn/ones issued inside pass 0 so wk/wv go first on this ring)

        # ---- resident weights / activations ----
        wq_sb = [
            wp.tile([128, c.DT, 4 * c.HD], BF16, name=f"wq{g}", tag=f"wq{g}")
            for g in range(2)
        ]
        wk_sb = wp.tile([128, c.DT, 2 * c.HD], BF16, name="wk_sb")
        wv_sb = wp.tile([128, c.DT, 2 * c.HD], BF16, name="wv_sb")

        q_bf = [
            qkp.tile([128, c.S], BF16, name=f"q{h}", tag=f"q{h}") for h in range(8)
        ]
        k_bf = [
            qkp.tile([128, c.S], BF16, name=f"k{g}", tag=f"k{g}") for g in range(2)
        ]
        vt = [
            qkp.tile([128, c.S], BF16, name=f"vt{g}", tag=f"vt{g}") for g in range(2)
        ]
        vn = [
            qkp.tile([128, c.KT, c.HD], BF16, name=f"vn{g}", tag=f"vn{g}")
            for g in range(2)
        ]
        ats = [
            qkp.tile([128, c.S], BF16, name=f"at{h}", tag=f"at{h}") for h in range(8)
        ]

        # ================= Phase 1: projections (+rope, V transpose) ========
        # Drains for s-chunk sc are emitted at the TOP of the next chunk's
        # loop (before the PSUM accumulators are re-allocated) so the WAR
        # dependencies are visible to the tile framework. First-read copies
        # are spread across Scalar/DVE; the pair-swap matmuls then give the
        # PE immediate work while the copies drain.
        pending_rope = []      # deferred rope-chain closures
        final_chain = [False]  # True while flushing the last pass's chains

        def emit_drains(g, sc, sl, accs):
            # PSUM->SBUF copies (bf16) happen immediately to free the banks;
            # Q heads drain as PAIRS ([128,1024] per instruction) to amortize
            # the ~350-cycle fixed cost of Act/DVE instructions.
            tq = []
            for i in range(2):
                t = rtp.tile([128, 2 * c.QCH], BF16, name=f"t{g}{sc}{i}",
                             tag="rt2", bufs=3)
                if i == 0:
                    nc.scalar.copy(t[:], accs[0])   # pa: q0,q1
                else:
                    nc.vector.tensor_copy(t[:], accs[1])  # pb: q2,q3
                tq.append(t)
            ts = [tq[0][:, :c.QCH], tq[0][:, c.QCH:],
                  tq[1][:, :c.QCH], tq[1][:, c.QCH:]]
            tk = rtp.tile([128, c.QCH], BF16, name=f"tk{g}{sc}", tag="rt", bufs=2)
            nc.scalar.copy(tk[:], accs[2])
            ts.append(tk[:])
            # Pool cannot read PSUM; V drain goes on DVE (casts to bf16)
            nc.vector.tensor_copy(vt[g][:, sl], accs[3])
            # rope the 5 copied tensors (4 Q + K), all-bf16 pipeline
            dsts = [q_bf[g * 4 + i] for i in range(4)] + [k_bf[g]]
            for i, (t, dst) in enumerate(zip(ts, dsts)):
                def chain(i=i, t=t, dst=dst, g=g, sc=sc, sl=sl):
                    rps = p_zz(f"rps{g}{sc}{i}")
                    nc.tensor.matmul(rps[:], pm_sb[:], t)
                    sw = rtp.tile([128, c.QCH], BF16, name=f"sw{g}{sc}{i}",
                                  tag="sw", bufs=2)
                    nc.vector.tensor_tensor(sw[:], rps[:], rb_sb[:, sl], MUL)
                    tr = rtp.tile([128, c.QCH], BF16, name=f"tr{g}{sc}{i}",
                                  tag="tr", bufs=2)
                    nc.gpsimd.tensor_tensor(tr[:], t, ra_sb[:, sl], MUL)
                    eng = nc.gpsimd if final_chain[0] else nc.vector
                    eng.tensor_tensor(dst[:, sl], tr[:], sw[:], ADD)
                pending_rope.append(chain)

        def make_transposes(g):
            # transpose QUADS of k-tiles into one [128,512] PSUM tile so the
            # vn drain is one Act copy per quad (amortizes instruction cost)
            out = []
            for st_i in range(0, c.KT, 4):
                def tr_one(st_i=st_i, g=g):
                    tp = p_zz(f"tp{g}{st_i}", [128, 512], BF16)
                    for j in range(4):
                        s_ = st_i + j
                        nc.tensor.transpose(
                            tp[:, j * 128:(j + 1) * 128],
                            vt[g][:, s_ * 128:(s_ + 1) * 128], idn_sb[:]
                        )
                    nc.scalar.copy(vn[g][:, st_i:st_i + 4, :], tp[:])
                out.append(tr_one)
            return out

        drainq = []            # deferred (g, sc, sl, accs), depth 2
        pending_tr = []        # deferred V-transpose closures

        def pop_drain():
            dg, dsc, dsl, daccs = drainq.pop(0)
            emit_drains(dg, dsc, dsl, daccs)
            if dsc == c.NQC - 1:  # group dg's V is complete
                pending_tr.extend(make_transposes(dg))

        for g in range(2):
            for sc in range(c.NQC):
                sl = slice(sc * c.QCH, (sc + 1) * c.QCH)
                if len(drainq) >= 1:
                    pop_drain()
                pa = p_stp2(f"pa{g}{sc}")
                pb = p_stp2(f"pb{g}{sc}")
                qaccs = [pa[:, :c.QCH], pa[:, c.QCH:], pb[:, :c.QCH],
                         pb[:, c.QCH:]]
                accs = [pa[:], pb[:], p_ot(f"ak{g}{sc}")[:],
                        p_ot(f"av{g}{sc}")[:]]
                # merged x DMAs: 4 d-tiles per issue
                xts = []
                for dq in range(c.DT // 4):
                    xt_t = xp.tile([128, 4, c.QCH], BF16, name=f"xt{dq}",
                                   tag="xt")
                    xts.append(xt_t)
                for dt in range(c.DT):
                    dq, dr_ = dt // 4, dt % 4
                    if dr_ == 0:
                        # x rides Sync; wq quarters ride the Scalar queue and
                        # wk/wv quarters the Pool queue, so the three DMA
                        # rings transfer in parallel at startup. The first
                        # group is split finer so dt 0 starts sooner.
                        ds_ = slice(4 * dq, 4 * dq + 4)
                        if g == 0 and sc == 0:
                            if dq == 0:
                                nc.sync.dma_start(xts[0][:, 0:1, :],
                                                  xt_d[0, 0, :, 0:1, :])
                                nc.sync.dma_start(xts[0][:, 1:2, :],
                                                  xt_d[0, 0, :, 1:2, :])
                                nc.sync.dma_start(xts[0][:, 2:4, :],
                                                  xt_d[0, 0, :, 2:4, :])
                                nc.scalar.dma_start(
                                    wq_sb[0][:, 0:1, :], wq_d[0, :, 0:1, :])
                                nc.scalar.dma_start(
                                    wq_sb[0][:, 1:2, :], wq_d[0, :, 1:2, :])
                                nc.scalar.dma_start(
                                    wq_sb[0][:, 2:4, :], wq_d[0, :, 2:4, :])
                            else:
                                nc.sync.dma_start(xts[dq][:], xt_d[sc, dq])
                                nc.scalar.dma_start(
                                    wq_sb[0][:, ds_, :], wq_d[0, :, ds_, :])
                            nc.gpsimd.dma_start(
                                wk_sb[:, ds_, :], wk_d[:, ds_, :])
                            nc.gpsimd.dma_start(
                                wv_sb[:, ds_, :], wv_d[:, ds_, :])
                        else:
                            nc.sync.dma_start(xts[dq][:], xt_d[sc, dq])
                            # group-1 wq arrives during passes 2-3: clear of
                            # the bandwidth-critical startup window, well
                            # before its first use in pass 4
                            if g == 0 and sc in (2, 3) and dq < 2:
                                ds2 = slice(8 * (sc - 2) + 4 * dq,
                                            8 * (sc - 2) + 4 * dq + 4)
                                nc.scalar.dma_start(
                                    wq_sb[1][:, ds2, :], wq_d[1, :, ds2, :])
                    xt_t = xts[dq][:, dr_, :]
                    st, sp = dt == 0, dt == c.DT - 1
                    for i in range(4):
                        nc.tensor.matmul(
                            qaccs[i],
                            wq_sb[g][:, dt, i * c.HD:(i + 1) * c.HD],
                            xt_t, start=st, stop=sp,
                        )
                    nc.tensor.matmul(
                        accs[2], wk_sb[:, dt, g * c.HD:(g + 1) * c.HD],
                        xt_t, start=st, stop=sp,
                    )
                    nc.tensor.matmul(
                        accs[3], wv_sb[:, dt, g * c.HD:(g + 1) * c.HD],
                        xt_t, start=st, stop=sp,
                    )
                    # the big rope/mask tables ride the Pool DMA queue after
                    # the startup-critical weights; the score-pair buffers
                    # are scrubbed once so the masked multiply of stale data
                    # can never hit an inf/nan left over in SBUF
                    if g == 0 and sc == 0:
                        if dt == 2:
                            nc.gpsimd.dma_start(pm_sb[:], pm_d[:])
                            nc.gpsimd.dma_start(idn_sb[:], idn_d[:])
                            nc.gpsimd.dma_start(ones_sb[:], ones_d[:])
                        elif dt == 7:
                            # rb feeds the first rope op (DVE sw); ra rides
                            # the Scalar ring which is idle by now
                            nc.gpsimd.dma_start(rb_sb[:], rb_d[:])
                            nc.scalar.dma_start(ra_sb[:], ra_d[:])
                        elif dt == 15:
                            nc.gpsimd.dma_start(cm_sb[:], cm_d[:])
                            for i_ in range(4):
                                t_ = ptp.tile([128, 2 * c.QCH], BF16,
                                              name=f"pti{i_}", tag="pt2")
                                nc.gpsimd.memset(t_[:], 0.0)
                    # interleave deferred rope chains and V transposes
                    if pending_rope and dt % 3 == 1:
                        pending_rope.pop(0)()
                    if pending_tr and dt % 2 == 0 and dt > 0:
                        pending_tr.pop(0)()
                drainq.append((g, sc, sl, accs))
        while drainq:
            pop_drain()
        # the final flush's chains land on the Act/DVE-bound first attention
        # set; their q_bf adds are slack-rich (only qc3 reads them), so they
        # ride the otherwise-idle Pool engine
        final_chain[0] = True
        while pending_rope:
            pending_rope.pop(0)()
        final_chain[0] = False

        # ================= Phase 3 chunk generator ==========================
        # P3 computes oT[d, q] = sum over all 8 heads of wo[hd, d] * at_h[hd,
        # q]; a (dt, qc) chunk becomes eligible once every head's qc-slice of
        # at is normalized, i.e. after attention qc-set qc completes.
        wo_tiles = {}          # fetch_i -> live tile (refetched per qc-set)
        wo_order = []          # (dt) fetch order
        wo_fetch_idx = [0]

        def wo_dma_next(dual=False):
            # wo fetches ride the Pool queue: Sync is saturated by the
            # output writes during P3, and Pool's reads all complete well
            # before the end-of-kernel drain. In the endgame (Act is idle)
            # the two halves go on separate rings for bandwidth.
            i = wo_fetch_idx[0]
            if i < len(wo_order):
                dt = wo_order[i]
                wt = wop.tile([128, 8 * c.HD], BF16, name=f"wo_t{i}", tag="wo")
                if dual:
                    nc.gpsimd.dma_start(wt[:, :4 * c.HD], wo_d[dt, :, :4 * c.HD])
                    nc.scalar.dma_start(wt[:, 4 * c.HD:], wo_d[dt, :, 4 * c.HD:])
                else:
                    nc.gpsimd.dma_start(wt[:], wo_d[dt])
                wo_tiles[i] = wt
                wo_fetch_idx[0] += 1

        p3_emitted = [0]

        def make_p3_set(qc, endgame=False):
            """16 closures, one per dt, each computing output tile (dt, qc)."""
            chunks = []
            qsl = slice(qc * c.QCH, (qc + 1) * c.QCH)
            for dt in range(c.DT):
                wo_order.append(dt)

                def chunk(dt=dt, qc=qc, fetch_i=len(wo_order) - 1,
                          endgame=endgame):
                    while fetch_i >= wo_fetch_idx[0]:
                        wo_dma_next(dual=endgame)
                    wt = wo_tiles.pop(fetch_i)
                    # prefetch up to three tiles ahead (wop holds 4)
                    tgt = min(fetch_i + 4, len(wo_order))
                    while wo_fetch_idx[0] < tgt:
                        wo_dma_next(dual=endgame)
                    idx = p3_emitted[0]
                    p3_emitted[0] += 1
                    if endgame and idx % 2 == 1:
                        oT = p_ot(f"oT{qc}{dt}")
                    else:
                        oT = p_zz(f"oT{qc}{dt}")
                    for j in range(8):
                        nc.tensor.matmul(
                            oT[:],
                            wt[:, j * c.HD:(j + 1) * c.HD],
                            ats[j][:, qsl],
                            start=(j == 0), stop=(j == 7),
                        )
                    oc = ocp.tile([128, c.QCH], BF16, name="oc", tag="oc")
                    if idx == 63:
                        # final chunk: drain halves on both engines + two
                        # parallel DMAs to shorten the kernel epilogue
                        nc.scalar.copy(oc[:, :c.QCH // 2], oT[:, :c.QCH // 2])
                        nc.vector.tensor_copy(oc[:, c.QCH // 2:],
                                              oT[:, c.QCH // 2:])
                        half = c.QCH // 2
                        q0 = qc * c.QCH
                        nc.sync.dma_start(
                            out_d[dt * 128:(dt + 1) * 128, q0:q0 + half],
                            oc[:, :half])
                        nc.sync.dma_start(
                            out_d[dt * 128:(dt + 1) * 128, q0 + half:q0 + c.QCH],
                            oc[:, half:])
                        return
                    if (idx // 2) % 2 == 0:
                        nc.scalar.copy(oc[:], oT[:])
                    else:
                        nc.vector.tensor_copy(oc[:], oT[:])
                    nc.sync.dma_start(
                        out_d[dt * 128:(dt + 1) * 128, qsl], oc[:]
                    )
                chunks.append(chunk)
            return chunks

        # ================= Phase 2: causal attention (qc-outer) =============
        # Scores are computed in PAIRS: one [128,1024] PSUM tile = two k-tiles
        # for the same q-chunk; exp handles both in one Activation (except
        # the diagonal-most pair, split to skip sub-band columns).
        for _ in range(2):
            wo_dma_next()
        p3_fifo = []

        prev_tail = None       # (closure, qc) of the previous block's tail
        blocks = [(qc, g, h) for qc in range(c.NQC) for g in range(2)
                  for h in range(4)]
        # interleave density: P3 chunks to pop per block, by qc-set
        p3_per_block = {0: [0] * 8, 1: [3, 3, 3, 3, 1, 1, 1, 1],
                        2: [3, 3, 3, 3, 1, 1, 1, 1],
                        3: [3, 3, 3, 3, 1, 1, 1, 1]}
        for bi, (qc, g, h) in enumerate(blocks):
            # group 1's V transposes drip in during the first attention sets
            for _ in range(2):
                if pending_tr:
                    pending_tr.pop(0)()
            qh = g * 4 + h
            qsl = slice(qc * c.QCH, (qc + 1) * c.QCH)
            nkt = c.RB * (qc + 1)
            npair = nkt // 2
            ot = p_ot(f"ot{qh}{qc}")
            if qc == 0:
                rsp = None
                zbp0 = p_zz(f"zb0{qh}{qc}")
            else:
                rsp = rp.tile([128, 2 * c.QCH], BF16, name=f"rs{qh}{qc}",
                              tag="rs", bufs=2)
                zbp0 = None
            av_q = []
            budget = p3_per_block[qc][bi % 8]
            # front-load one P3 chunk at block top: independent PE work
            # while the previous block's diagonal pairs drain through exp
            if p3_fifo and budget > 0 and bi % 2 == 0:
                p3_fifo.pop(0)()
                budget -= 1
            for pk in range(npair):
                # independent PE work (previous tail, P3 chunks) goes in
                # FRONT of the next score matmuls, which may block on the
                # exp of pair pk-2 (PSUM bank WAR)
                if pk == 1 and prev_tail is not None:
                    tail_fn, tail_qc = prev_tail
                    tail_fn()
                    prev_tail = None
                    if tail_qc != qc:  # finished attention qc-set tail_qc
                        p3_fifo.extend(make_p3_set(tail_qc))
                    if p3_fifo and budget > 0:
                        p3_fifo.pop(0)()
                        budget -= 1
                elif pk > 1 and pk % 2 == 1 and p3_fifo and budget > 0:
                    p3_fifo.pop(0)()
                    budget -= 1
                sp2 = p_stp2(f"sp{qh}{qc}{pk}")
                for half in range(2):
                    kt = 2 * pk + half
                    ridx = kt - (nkt - c.RB)
                    qlo = max(ridx, 0) * 128  # cols below the band are masked
                    o = half * c.QCH
                    nc.tensor.matmul(
                        sp2[:, o + qlo:o + c.QCH],
                        k_bf[g][:, kt * 128:(kt + 1) * 128],
                        q_bf[qh][:, qc * c.QCH + qlo:(qc + 1) * c.QCH],
                    )
                pt2 = ptp.tile([128, 2 * c.QCH], BF16, name="pt2", tag="pt2")
                if pk == npair - 1:
                    # diagonal-most pair: halves have ridx 2 and 3; exp only
                    # the in-band columns (the mask zeroes the stale rest)
                    nc.scalar.activation(pt2[:, 256:c.QCH], sp2[:, 256:c.QCH],
                                         AF.Exp, scale=scale)
                    nc.scalar.activation(pt2[:, c.QCH + 384:], sp2[:, c.QCH + 384:],
                                         AF.Exp, scale=scale)
                else:
                    nc.scalar.activation(pt2[:], sp2[:], AF.Exp, scale=scale)
                # causal mask: one full-width multiply per band pair (the cm
                # table is 1 outside the triangles, 0 below the band)
                pi = pk - (npair - 2)
                if pi >= 0:
                    nc.vector.tensor_tensor(
                        pt2[:], pt2[:],
                        cm_sb[:, 2 * pi * c.QCH:(2 * pi + 2) * c.QCH], MUL
                    )
                if qc == 0:
                    # PE idles in the first set: accumulate the denominator
                    # broadcast directly with ones-matmuls instead of DVE
                    # partial-sum adds
                    for half in range(2):
                        kt = 2 * pk + half
                        nc.tensor.matmul(
                            zbp0[:], ones_sb[:],
                            pt2[:, half * c.QCH:(half + 1) * c.QCH],
                            start=(kt == 0), stop=(kt == nkt - 1),
                        )
                elif pk == 0:
                    nc.vector.tensor_copy(rsp[:], pt2[:])
                else:
                    nc.vector.tensor_tensor(rsp[:], rsp[:], pt2[:], ADD)
                def emit_av(k2, p2, nkt=nkt, g=g, ot=ot):
                    for half in range(2):
                        kt = 2 * k2 + half
                        ridx = kt - (nkt - c.RB)
                        qlo = max(ridx, 0) * 128
                        o = half * c.QCH
                        nc.tensor.matmul(
                            ot[:, qlo:], vn[g][:, kt, :],
                            p2[:, o + qlo:o + c.QCH],
                            start=(kt == 0), stop=(kt == nkt - 1),
                        )

                av_q.append((pk, pt2))
                if len(av_q) > 2:
                    emit_av(*av_q.pop(0))
            for k2, p2 in av_q:
                emit_av(k2, p2)
            while budget > 0 and p3_fifo:
                p3_fifo.pop(0)()
                budget -= 1

            def make_tail(ot=ot, rsp=rsp, zbp0=zbp0, qh=qh, qsl=qsl, qc=qc):
                def tail():
                    # denominator broadcast: all-ones stationary sums the
                    # partition dim; every output partition gets the sum
                    if zbp0 is not None:
                        zbp = zbp0
                    else:
                        zbp = p_zz(f"zb{qh}{qc}")
                        nc.tensor.matmul(zbp[:], ones_sb[:], rsp[:, :c.QCH],
                                         start=True, stop=False)
                        nc.tensor.matmul(zbp[:], ones_sb[:], rsp[:, c.QCH:],
                                         start=False, stop=True)
                    zr = rp.tile([128, c.QCH], F32, name=f"zr{qh}{qc}",
                                 tag="zr", bufs=2)
                    nc.vector.reciprocal_approx_fast(zr[:], zbp[:])
                    nc.vector.tensor_tensor(ats[qh][:, qsl], ot[:], zr[:], MUL)
                return tail

            prev_tail = (make_tail(), qc)
        prev_tail[0]()
        p3_fifo.extend(make_p3_set(3, endgame=True))

        # ================= Phase 3 remainder ================================
        for i, ch in enumerate(p3_fifo):
            ch()
        # re-tag remaining chunks as endgame for PSUM alternation is handled
        # inside make_p3_set; nothing further

    nc.compile()
    nc.finalize()
    return nc


# ---------------------------------------------------------------------------
# Host-side sharding / gathering
# ---------------------------------------------------------------------------

def host_prep(x, freq_cis, wq, wk, wv, wo, n_cores, cfg: Cfg):
    import ml_dtypes
    BF = ml_dtypes.bfloat16
    c = cfg
    HD, HQC = c.HD, c.HQC

    x = np.asarray(x, np.float32)
    freq_cis = np.asarray(freq_cis, np.float32)
    wq = np.asarray(wq, np.float32)
    wk = np.asarray(wk, np.float32)
    wv = np.asarray(wv, np.float32)
    wo = np.asarray(wo, np.float32)

    # rope tables, interleaved layout: out[p] = ra[p]*t[p] + rb[p]*t[partner(p)]
    a = freq_cis[:, :, 0, 0].T
    bb = freq_cis[:, :, 0, 1].T
    cc = freq_cis[:, :, 1, 0].T
    dd = freq_cis[:, :, 1, 1].T
    S_ = freq_cis.shape[0]
    ra = np.empty((HD, S_), np.float32)
    rb = np.empty((HD, S_), np.float32)
    ra[0::2], ra[1::2] = a, dd
    rb[0::2], rb[1::2] = bb, cc
    ra = ra.astype(BF)
    rb = rb.astype(BF)

    pm = np.zeros((HD, HD), np.float32)
    idx = np.arange(HD)
    pm[idx, idx ^ 1] = 1.0
    pm = pm.astype(BF)

    # causal band masks: cm[k, m, q] = 1 if (k + 128*m) <= q
    ks = np.arange(128)[:, None]
    qs = np.arange(c.QCH)[None, :]
    cm = np.stack(
        [(ks + 128 * m <= qs) for m in range(c.RB)], axis=1
    ).astype(BF).reshape(128, c.RB * c.QCH)

    in_maps = []
    for core in range(n_cores):
        b, hh = core // 2, core % 2
        hq0 = hh * HQC
        # xt[sc, dq, p, t, s'] = x[b][sc*512+s', (4dq+t)*128+p]
        xt = np.ascontiguousarray(
            x[b].astype(BF).reshape(c.NQC, c.QCH, c.DT // 4, 4, 128)
            .transpose(0, 2, 4, 3, 1))

        wq_c = wq[hq0 * HD:(hq0 + HQC) * HD]              # [1024, D]
        # wq_p[g, p, dt, o] = wq_c[g*512+o, dt*128+p]
        wq_p = np.ascontiguousarray(
            wq_c.reshape(2, 4 * HD, c.DT, 128).transpose(0, 3, 2, 1).astype(BF)
        )
        wk_c = wk[2 * hh * HD:(2 * hh + 2) * HD]           # [256, D]
        # wk_p[p, dt, o] = wk_c[o, dt*128+p]
        wk_p = np.ascontiguousarray(
            wk_c.reshape(2 * HD, c.DT, 128).transpose(2, 1, 0).astype(BF))
        wv_c = wv[2 * hh * HD:(2 * hh + 2) * HD]
        wv_p = np.ascontiguousarray(
            wv_c.reshape(2 * HD, c.DT, 128).transpose(2, 1, 0).astype(BF))
        wo_c = wo[:, hq0 * HD:(hq0 + HQC) * HD]            # [D, 1024]
        # [DT, 128, 1024]: wo_p[dt, p, j*128+m] = wo_c[dt*128+m, j*128+p]
        wo_p = np.ascontiguousarray(
            wo_c.reshape(c.DT, 128, 8, 128).transpose(0, 3, 2, 1)
            .reshape(c.DT, 128, 8 * HD).astype(BF)
        )
        in_maps.append({
            "xt": xt,
            "wq": wq_p,
            "wk": wk_p,
            "wv": wv_p,
            "wo": wo_p,
            "ra": ra,
            "rb": rb,
            "cm": cm,
            "pm": pm,
            "idn": np.eye(128, dtype=BF),
            "ones": np.ones((128, 128), BF),
        })
    return in_maps


def run(inputs: dict, n_cores: int = 8, cfg: Cfg = Cfg(), trace: bool = False):
    in_maps = host_prep(
        inputs["x"], inputs["freq_cis"], inputs["wq"], inputs["wk"],
        inputs["wv"], inputs["wo"], n_cores, cfg,
    )
    nc = build_program(cfg)
    res = run_bass_kernel_spmd(nc, in_maps, list(range(n_cores)), trace=trace)
    out = np.empty((cfg.B, cfg.S, cfg.D), np.float32)
    for b in range(cfg.B):
        p0 = np.asarray(res.results[2 * b]["partialT"]).astype(np.float32)
        p1 = np.asarray(res.results[2 * b + 1]["partialT"]).astype(np.float32)
        out[b] = (p0 + p1).T
    return out, res


def kernel(**inputs) -> np.ndarray:
    out, _ = run(inputs, n_cores=8, cfg=Cfg())
    return out
